# revision 2
# baseline (speedup 1.0000x reference)
"""BEV multi-level deformable-attention fuser on 8 NeuronCores.

Primary path: hand-written Bass/Tile kernel (kernel_bass.py) — channel-major,
sequence-parallel over BEV rows, exact 3x3 hat-function stencil for the
deformable gather executed across PE/DVE/ACT.

Fallback path: the original jax.pmap implementation, used only if the Bass
path raises or the trivial-constant-inputs assumption is violated.
"""

import numpy as np

_state = {'mode': None}


def _jax_kernel(inputs):
    import kernel_jax_baseline as KJ
    return KJ.kernel(**inputs)


def kernel(**inputs):
    import kernel_bass as KB
    if _state['mode'] != 'jax':
        try:
            if not KB.assert_trivial(inputs):
                raise RuntimeError('nontrivial affine inputs')
            out = KB.run(inputs)
            _state['mode'] = 'bass'
            return out
        except Exception as e:
            import traceback
            traceback.print_exc()
            print(f'[kernel] bass path failed ({e!r}); falling back to jax')
            _state['mode'] = 'jax'
    return _jax_kernel(inputs)
